# revision 31
# baseline (speedup 1.0000x reference)
"""DynamicUncertaintyGCN Trainium2 kernel (8 NeuronCores, SPMD, NO collectives).

v2 design (vs v1 which used AllReduce/AllGather):
 - Every core receives the FULL fea (own slab as `fea_own`, the other 7 slabs
   as `fea_rest`) and computes the batch sum S = sum_b fea_b ON DEVICE while
   streaming (DVE/gpsimd adds chase the HWDGE DMAs). The graph build
   (cdist+topk) is fully replicated per core -> ZERO collectives, so core 0's
   measured exec window can never include cross-core launch skew.
 - Distance z = -2G + r_j + r_i is accumulated entirely in PSUM: 2 fp32
   Gram matmuls + two K=1 matmuls that broadcast (-r/2) along rows/columns.
 - d ~= 2*sqrt(z) via ACT-Sqrt + reciprocal_approx_fast (18-bit) NR form:
   d = y + z*recip(y); wide [128,1536] ops batched over 4 node tiles.
 - top-8 via DVE max8 + per-row threshold (gc >= mx[:,7]) -> 0/1 indicator
   (self included once) + explicit +I self loop.
 - GCN norm dis_i*dis_j is folded into the band: BAND = indicator * DD where
   DD = outer(dis, dis) comes from a K=1 PE matmul; deg from band column sums.
 - Aggregation emits c-major output directly (stationary = h node-major
   slice, moving = band) -> no PE transposes; relu+bias on ACT from PSUM,
   residual add on DVE.
 - MLP head: U1 f32r, U2/U3 bf16, sigmoid, K=1 broadcast, fused (1+u)*fea.
"""
import sys
sys.path.insert(0, '/opt/trn_rl_repo')
import numpy as np

import concourse.bass as bass
import concourse.tile as tile
from concourse import bacc, mybir
from concourse.bass_utils import run_bass_kernel_spmd

F32 = mybir.dt.float32
F32R = mybir.dt.float32r
BF16 = mybir.dt.bfloat16
FP16 = mybir.dt.float16
AF = mybir.ActivationFunctionType
OP = mybir.AluOpType

NCORES = 8
B, C, HH, WW = 8, 256, 64, 64
N = HH * WW            # 4096
P = 128
NT = N // P            # 32 node tiles
BW = 3 * P             # 384 band width
EXT = P                # 128 pad columns each side
NE = N + 2 * EXT       # 4352
GRP = 4                # node tiles per wide-op group
NG = NT // GRP         # 8 groups
BWP = 512              # band width padded to one PSUM bank (matmul output
                       # regions must NOT cross 2KB PSUM bank boundaries)

_cache = {}
_runner_cache = {}


def _spatial07():
    """S07[p, c]: 0.7 * true 2D spatial distance for flat offset d = c-128-p,
    as a function of x = p % 64 only (y-independent; out-of-range columns are
    excluded by the poisoned rh_row pads)."""
    s = np.zeros((P, BW), np.float32)
    for p in range(P):
        x = p % WW
        for c in range(BW):
            d = c - P - p
            xs = x + d
            dyv = xs // WW
            dxv = (xs % WW) - x
            s2 = np.float32(dyv * dyv + dxv * dxv)
            s[p, c] = np.float32(0.7) * np.float32(np.sqrt(s2, dtype=np.float32))
    return s


def _build(reps=1, debug=False):
    nc = bacc.Bacc("TRN2", target_bir_lowering=False, debug=False,
                   enable_asserts=True, num_devices=NCORES)

    # ---- external I/O ----
    fea_own = nc.dram_tensor("fea_own", [C, N], F32R, kind="ExternalInput").ap()
    fea_all = nc.dram_tensor("fea_all", [B * C, N], F32,
                             kind="ExternalInput").ap()
    Wd = nc.dram_tensor("Wd", [3, C, C], F32R, kind="ExternalInput").ap()
    bd = nc.dram_tensor("bd", [3, C], F32, kind="ExternalInput").ap()
    U1d = nc.dram_tensor("U1d", [C, 128], F32R, kind="ExternalInput").ap()
    U2d = nc.dram_tensor("U2d", [128, 64], F32R, kind="ExternalInput").ap()
    U3d = nc.dram_tensor("U3d", [64, 1], F32R, kind="ExternalInput").ap()
    ub1d = nc.dram_tensor("ub1d", [128], F32, kind="ExternalInput").ap()
    ub2d = nc.dram_tensor("ub2d", [64], F32, kind="ExternalInput").ap()
    ub3d = nc.dram_tensor("ub3d", [1], F32, kind="ExternalInput").ap()
    out_d = nc.dram_tensor("out", [C, N], F32, kind="ExternalOutput").ap()
    if debug:
        dbg_dis = nc.dram_tensor("dbg_dis", [P, NT], F32, kind="ExternalOutput").ap()
        dbg_deg = nc.dram_tensor("dbg_deg", [P, NT], F32, kind="ExternalOutput").ap()
        dbg_und = nc.dram_tensor("dbg_und", [P, NT * BW], F32,
                                 kind="ExternalOutput").ap()
        dbg_comb = nc.dram_tensor("dbg_comb", [P, NT * BW], F32,
                                  kind="ExternalOutput").ap()
        dbg_x = nc.dram_tensor("dbg_x", [3, P, 2 * N], F32,
                               kind="ExternalOutput").ap()
        dbg_u = nc.dram_tensor("dbg_u", [1, N], F32, kind="ExternalOutput").ap()

    # ---- inline constants ----
    s07x4_c = nc.inline_tensor(np.tile(_spatial07(), (1, GRP)).copy(), name="s07x4c")
    diag = np.zeros((P, GRP * BW), np.float32)
    for p in range(P):
        for i in range(GRP):
            diag[p, i * BW + P + p] = 1.0
    diag4_c = nc.inline_tensor(diag, name="diag4c")
    idf32_c = nc.inline_tensor(np.eye(P, dtype=np.float32), name="idf32c")
    onesr_c = nc.inline_tensor(np.ones((1, P), np.float32), name="onesrc")
    ones384_c = nc.inline_tensor(np.ones((1, BW), np.float32), name="ones384c")
    onescol_c = nc.inline_tensor(np.ones((P, 1), np.float32), name="onescolc")

    with tile.TileContext(nc) as tc:
        with (
            tc.tile_pool(name="const", bufs=1) as cpool,
            tc.tile_pool(name="persist", bufs=1) as pp,
        ):
            # ---------- constants to SBUF ----------
            s07x4 = cpool.tile([P, GRP * BW], F32)
            nc.sync.dma_start(s07x4[:], s07x4_c.ap()[:])
            diag4 = cpool.tile([P, GRP * BW], F32)
            nc.scalar.dma_start(diag4[:], diag4_c.ap()[:])
            idf32 = cpool.tile([P, P], F32)
            nc.sync.dma_start(idf32[:], idf32_c.ap()[:])
            onesr1 = cpool.tile([1, P], F32)
            nc.scalar.dma_start(onesr1[:], onesr_c.ap()[:])
            onesr_r = cpool.tile([1, P], F32R)
            nc.gpsimd.dma_start(onesr_r[:], onesr_c.ap()[:])
            ones384 = cpool.tile([1, BW], F32)
            nc.sync.dma_start(ones384[:], ones384_c.ap()[:])
            onescol = cpool.tile([P, 1], F32)
            nc.scalar.dma_start(onescol[:], onescol_c.ap()[:])
            ones1b = cpool.tile([P, 1], FP16)
            nc.vector.memset(ones1b[:], 1.0)

            w_sb = cpool.tile([P, 3 * 2 * C], F32R)
            for l in range(3):
                for ct in range(2):
                    q = nc.sync if ct == 0 else nc.scalar
                    q.dma_start(w_sb[:, (l * 2 + ct) * C:(l * 2 + ct + 1) * C],
                                Wd[l, ct * P:(ct + 1) * P, :])
            b_sb = cpool.tile([P, 6], F32)
            for l in range(3):
                for ct in range(2):
                    nc.sync.dma_start(b_sb[:, l * 2 + ct:l * 2 + ct + 1],
                                      bd[l, ct * P:(ct + 1) * P][:, None])
            u1_sb = cpool.tile([P, 2 * 128], F32R)
            for ct in range(2):
                (nc.sync if ct == 0 else nc.scalar).dma_start(
                    u1_sb[:, ct * 128:(ct + 1) * 128],
                    U1d[ct * P:(ct + 1) * P, :])
            u2_sb = cpool.tile([P, 64], F32R)
            nc.scalar.dma_start(u2_sb[:], U2d[:])
            u3_sb = cpool.tile([64, 1], F32R)
            nc.sync.dma_start(u3_sb[:], U3d[:])
            ub1_sb = cpool.tile([P, 1], F32)
            nc.sync.dma_start(ub1_sb[:], ub1d[:, None])
            ub2_sb = cpool.tile([64, 1], F32)
            nc.scalar.dma_start(ub2_sb[:], ub2d[:, None])
            ub3_sb = cpool.tile([1, 1], F32)
            nc.sync.dma_start(ub3_sb[:], ub3d[:, None])

            # ---------- persistent tensors ----------
            x_cn = pp.tile([P, 2 * N], F32R)       # own batch, C-major, 2 c-tiles
            UND = pp.tile([P, NT * BW], FP16)      # band indicator -> scaled band
            h1raw = pp.tile([P, NT * C], FP16)     # layer-1 x@W1 (= h_all of l0)
            degcol = pp.tile([P, NT], F32)
            discol = pp.tile([P, NT], F32)

            for rep in range(reps):
                if rep > 0:
                    tc.strict_bb_all_engine_barrier()

                # =========== phase A: stream full fea, sum into S_ext ==========
                with tc.tile_pool(name="ag", bufs=1) as agp:
                    S_ext = agp.tile([P, 2 * NE], F32)
                    disrow = agp.tile([1, NE], F32)
                    rh_row = agp.tile([1, NE], F32)

                    nc.sync.dma_start(x_cn[:, 0:N], fea_own[0:P, :])
                    nc.scalar.dma_start(x_cn[:, N:2 * N], fea_own[P:C, :])

                    with (
                        tc.tile_pool(name="stream", bufs=2) as spool,
                        tc.tile_pool(name="h1ps", bufs=4, space="PSUM") as h1ps,
                        tc.tile_pool(name="rps", bufs=2, space="PSUM") as rps,
                    ):
                        # pads zero; stream ALL 8 slabs in global batch order
                        # (summation order must be IDENTICAL on every core so
                        # the replicated graphs agree bit-exactly)
                        for ct in range(2):
                            base = ct * NE
                            nc.vector.memset(S_ext[:, base:base + EXT], 0.0)
                            nc.vector.memset(S_ext[:, base + EXT + N:base + NE], 0.0)
                        for ct in range(2):
                            for b8 in range(B):
                                buf = spool.tile([P, N], F32, tag="sb")
                                q = nc.sync if ct == 0 else nc.scalar
                                q.dma_start(buf[:], fea_all[(b8 * 2 + ct) * P:
                                                            (b8 * 2 + ct + 1) * P, :])
                                eng = nc.gpsimd if ct == 0 else nc.vector
                                dst = S_ext[:, ct * NE + EXT:ct * NE + EXT + N]
                                if b8 == 0:
                                    eng.tensor_copy(dst, buf[:])
                                else:
                                    eng.tensor_add(dst, dst, buf[:])

                        # h1 precompute (PE/ACT under the DMA shadow)
                        for jb in range(NT):
                            hp1 = h1ps.tile([P, C], F32, space="PSUM", tag="hp1")
                            for ct in range(2):
                                nc.tensor.matmul(
                                    hp1[:],
                                    x_cn[:, ct * N + jb * P:ct * N + (jb + 1) * P],
                                    w_sb[:, ct * C:(ct + 1) * C],
                                    start=(ct == 0), stop=(ct == 1))
                            nc.scalar.activation(h1raw[:, jb * C:(jb + 1) * C],
                                                 hp1[:], AF.Copy)

                        # rh_row = -0.5 * column sums of S^2 (poisoned pads)
                        s2_0 = spool.tile([P, N], F32, tag="sb")
                        nc.gpsimd.tensor_mul(s2_0[:], S_ext[:, EXT:EXT + N],
                                             S_ext[:, EXT:EXT + N])
                        s2_1 = spool.tile([P, N], F32, tag="sb")
                        nc.vector.tensor_mul(s2_1[:], S_ext[:, NE + EXT:NE + EXT + N],
                                             S_ext[:, NE + EXT:NE + EXT + N])
                        for ch in range(8):
                            rput = rps.tile([1, 512], F32, space="PSUM", tag="rp")
                            nc.tensor.matmul(rput[:], onescol[:],
                                             s2_0[:, ch * 512:(ch + 1) * 512],
                                             start=True, stop=False)
                            nc.tensor.matmul(rput[:], onescol[:],
                                             s2_1[:, ch * 512:(ch + 1) * 512],
                                             start=False, stop=True)
                            nc.scalar.activation(
                                rh_row[0:1, EXT + ch * 512:EXT + (ch + 1) * 512],
                                rput[:], AF.Copy, scale=-0.5)
                        nc.vector.memset(rh_row[0:1, 0:EXT], -5e7)
                        nc.vector.memset(rh_row[0:1, EXT + N:NE], -5e7)
                        nc.gpsimd.memset(disrow[0:1, 0:EXT], 0.0)
                        nc.gpsimd.memset(disrow[0:1, EXT + N:NE], 0.0)

                    # =========== phase G: graph build (replicated) ===========
                    # PSUM layout: pg_t/dd_t are [P, 4*512] f32 (4 banks each);
                    # each node tile gets a 512-col bank-aligned region whose
                    # live part is [i*512, i*512+384) -- matmul outputs must
                    # never cross a 2KB PSUM bank boundary.  The 128-col gaps
                    # host deg psums (pg_t) and dis-row transposes (dd_t).
                    # SBUF scratch stays compact (384-stride) and is bridged
                    # to the padded PSUM with 3D strided APs.
                    with (
                        tc.tile_pool(name="gsc", bufs=2) as gsc,
                        tc.tile_pool(name="gps", bufs=1, space="PSUM") as gps,
                    ):
                        pg_t = gps.tile([P, GRP * BWP], F32, space="PSUM")
                        dd_t = gps.tile([P, GRP * BWP], F32, space="PSUM")
                        pg_3 = pg_t[:].rearrange("p (g w) -> p g w", w=BWP)[:, :, 0:BW]
                        dd_3 = dd_t[:].rearrange("p (g w) -> p g w", w=BWP)[:, :, 0:BW]

                        def emit_deg(j):
                            # deg + dis + disrow chunk for tiles 4j..4j+3
                            for k in range(GRP):
                                jb = 4 * j + k
                                contribs = [(jb + 1 - ch, ch) for ch in range(3)
                                            if 0 <= jb + 1 - ch < NT]
                                for ci, (t, ch) in enumerate(contribs):
                                    nc.tensor.matmul(
                                        pg_t[:, BW + k:BW + k + 1],
                                        UND[:, t * BW + ch * P:t * BW + (ch + 1) * P],
                                        ones1b[:],
                                        start=(ci == 0), stop=(ci == len(contribs) - 1))
                            nc.scalar.activation(degcol[:, 4 * j:4 * j + 4],
                                                 pg_t[:, BW:BW + 4], AF.Copy)
                            nc.vector.reciprocal(out=discol[:, 4 * j:4 * j + 4],
                                                 in_=degcol[:, 4 * j:4 * j + 4])
                            nc.scalar.activation(discol[:, 4 * j:4 * j + 4],
                                                 discol[:, 4 * j:4 * j + 4], AF.Sqrt)
                            for k in range(GRP):
                                nc.tensor.transpose(
                                    out=dd_t[0:1, k * BWP + BW:k * BWP + BW + P],
                                    in_=discol[:, 4 * j + k:4 * j + k + 1],
                                    identity=idf32[:])
                                nc.scalar.activation(
                                    disrow[0:1, EXT + (4 * j + k) * P:
                                           EXT + (4 * j + k + 1) * P],
                                    dd_t[0:1, k * BWP + BW:k * BWP + BW + P],
                                    AF.Copy)

                        def emit_dd(j):
                            # scale band tiles 4j..4j+3 by outer(dis_i, dis_j)
                            for i in range(GRP):
                                t = 4 * j + i
                                nc.tensor.matmul(
                                    dd_t[:, i * BWP:i * BWP + BW],
                                    disrow[0:1, EXT + t * P:EXT + (t + 1) * P],
                                    disrow[0:1, t * P:t * P + BW],
                                    start=True, stop=True)
                            u3 = UND[:, 4 * j * BW:(4 * j + 4) * BW].rearrange(
                                "p (g w) -> p g w", w=BW)
                            nc.vector.tensor_mul(u3, u3, dd_3)

                        for g in range(NG):
                            for i in range(GRP):
                                t = 4 * g + i
                                s0 = i * BWP
                                for ct in range(2):
                                    nc.tensor.matmul(
                                        pg_t[:, s0:s0 + BW],
                                        S_ext[:, ct * NE + EXT + t * P:
                                              ct * NE + EXT + (t + 1) * P],
                                        S_ext[:, ct * NE + t * P:ct * NE + t * P + BW],
                                        start=(ct == 0), stop=False)
                                nc.tensor.matmul(
                                    pg_t[:, s0:s0 + BW], onesr1[:],
                                    rh_row[0:1, t * P:t * P + BW],
                                    start=False, stop=False)
                                nc.tensor.matmul(
                                    pg_t[:, s0:s0 + BW],
                                    rh_row[0:1, EXT + t * P:EXT + (t + 1) * P],
                                    ones384[:],
                                    start=False, stop=True)
                            # z = max(-2*P, eps); d = y + z*recip(y), y = sqrt(z)
                            gz = gsc.tile([P, GRP * BW], F32, tag="gz")
                            gz3 = gz[:].rearrange("p (g w) -> p g w", w=BW)
                            nc.vector.tensor_scalar(out=gz3, in0=pg_3,
                                                    scalar1=-2.0, scalar2=1e-8,
                                                    op0=OP.mult, op1=OP.max)
                            gy = gsc.tile([P, GRP * BW], F32, tag="gy")
                            nc.scalar.activation(gy[:], gz[:], AF.Sqrt)
                            gr = gsc.tile([P, GRP * BW], F32, tag="gr")
                            nc.vector.reciprocal_approx_fast(out=gr[:], in_=gy[:])
                            nc.gpsimd.tensor_mul(gz[:], gz[:], gr[:])   # z*r
                            nc.gpsimd.tensor_add(gz[:], gy[:], gz[:])   # d
                            # comb_neg = -(0.3/16)*d - s07
                            nc.vector.scalar_tensor_tensor(
                                out=gy[:], in0=gz[:], scalar=-0.01875, in1=s07x4[:],
                                op0=OP.mult, op1=OP.subtract)
                            if debug:
                                nc.sync.dma_start(
                                    dbg_comb[:, 4 * g * BW:(4 * g + 4) * BW], gy[:])
                            mxg = gsc.tile([P, 8 * GRP], F32, tag="mx")
                            for i in range(GRP):
                                nc.vector.max(out=mxg[:, 8 * i:8 * i + 8],
                                              in_=gy[:, i * BW:(i + 1) * BW])
                                nc.vector.scalar_tensor_tensor(
                                    out=UND[:, (4 * g + i) * BW:(4 * g + i + 1) * BW],
                                    in0=gy[:, i * BW:(i + 1) * BW],
                                    scalar=mxg[:, 8 * i + 7:8 * i + 8],
                                    in1=diag4[:, i * BW:(i + 1) * BW],
                                    op0=OP.is_ge, op1=OP.add)
                            if g >= 1:
                                emit_deg(g - 1)
                            if g >= 2:
                                emit_dd(g - 2)
                        emit_deg(NG - 1)
                        emit_dd(NG - 2)
                        emit_dd(NG - 1)
                        if debug:
                            nc.sync.dma_start(dbg_dis[:], discol[:])
                            nc.sync.dma_start(dbg_deg[:], degcol[:])
                            nc.gpsimd.dma_start(dbg_und[:], UND[:])

                # =========== phase L: 3 GCN layers ===========
                with tc.tile_pool(name="lsb", bufs=1) as lsb:
                    hall = lsb.tile([P, NT * C], FP16)
                    fea_sb = lsb.tile([P, 2 * N], F32R)
                    nc.sync.dma_start(fea_sb[:, 0:N], fea_own[0:P, :])
                    nc.scalar.dma_start(fea_sb[:, N:2 * N], fea_own[P:C, :])

                    with (
                        tc.tile_pool(name="lsc", bufs=4) as lsc,
                        tc.tile_pool(name="lps", bufs=4, space="PSUM") as lps,
                        tc.tile_pool(name="lps2", bufs=4, space="PSUM") as lps2,
                    ):
                      for l in range(3):
                        hsrc = h1raw if l == 0 else hall
                        if l > 0:
                            for jb in range(NT):
                                hp = lps.tile([P, C], F32, space="PSUM", tag="hp")
                                for ct in range(2):
                                    nc.tensor.matmul(
                                        hp[:],
                                        x_cn[:, ct * N + jb * P:ct * N + (jb + 1) * P],
                                        w_sb[:, (l * 2 + ct) * C:(l * 2 + ct + 1) * C],
                                        start=(ct == 0), stop=(ct == 1))
                                nc.scalar.activation(hall[:, jb * C:(jb + 1) * C],
                                                     hp[:], AF.Copy)
                        for ct in range(2):
                            for bk in range(8):
                                zp = lps2.tile([P, 512], F32, space="PSUM", tag="zp")
                                nc.vector.memset(zp[:], 0.0)
                                ts_list = [t for t in range(4 * bk - 1, 4 * bk + 5)
                                           if 0 <= t < NT]
                                for ti, t in enumerate(ts_list):
                                    lo = max(128 * (t - 1), 512 * bk)
                                    hi = min(128 * (t + 2), 512 * bk + 512)
                                    w0 = lo - 128 * (t - 1)
                                    p0 = lo - 512 * bk
                                    nc.tensor.matmul(
                                        zp[:, p0:p0 + (hi - lo)],
                                        hsrc[:, t * C + ct * P:t * C + ct * P + P],
                                        UND[:, t * BW + w0:t * BW + w0 + (hi - lo)],
                                        start=False, stop=(ti == len(ts_list) - 1),
                                        skip_group_check=True)
                                zr = lsc.tile([P, 512], F32, tag="zr")
                                nc.scalar.activation(zr[:], zp[:], AF.Relu,
                                                     bias=b_sb[:, l * 2 + ct:
                                                               l * 2 + ct + 1])
                                nc.vector.tensor_add(
                                    x_cn[:, ct * N + 512 * bk:ct * N + 512 * (bk + 1)],
                                    x_cn[:, ct * N + 512 * bk:ct * N + 512 * (bk + 1)],
                                    zr[:])
                        if debug:
                            nc.gpsimd.dma_start(dbg_x[l, :, :], x_cn[:])

                    # =========== phase M: MLP head + output ===========
                    with (
                        tc.tile_pool(name="msc", bufs=3) as msc,
                        tc.tile_pool(name="mps", bufs=2, space="PSUM") as mps,
                    ):
                        z1 = lsb.tile([P, N], F32R)
                        for ch in range(8):
                            zpm = mps.tile([P, 512], F32, space="PSUM", tag="zpm")
                            for ct in range(2):
                                nc.tensor.matmul(
                                    zpm[:], u1_sb[:, ct * 128:(ct + 1) * 128],
                                    x_cn[:, ct * N + ch * 512:ct * N + (ch + 1) * 512],
                                    start=(ct == 0), stop=(ct == 1))
                            nc.scalar.activation(z1[:, ch * 512:(ch + 1) * 512],
                                                 zpm[:], AF.Gelu, bias=ub1_sb[:])
                        z2 = lsb.tile([64, N], F32R)
                        for ch in range(8):
                            zp2 = mps.tile([64, 512], F32, space="PSUM", tag="zp2")
                            nc.tensor.matmul(zp2[:], u2_sb[:],
                                             z1[:, ch * 512:(ch + 1) * 512],
                                             start=True, stop=True)
                            nc.scalar.activation(z2[:, ch * 512:(ch + 1) * 512],
                                                 zp2[:], AF.Gelu, bias=ub2_sb[:])
                        u_row = lsb.tile([1, N], F32R)
                        for ch in range(8):
                            up = mps.tile([1, 512], F32, space="PSUM", tag="up")
                            nc.tensor.matmul(up[:], u3_sb[:],
                                             z2[:, ch * 512:(ch + 1) * 512],
                                             start=True, stop=True)
                            nc.scalar.activation(u_row[:, ch * 512:(ch + 1) * 512],
                                                 up[:], AF.Sigmoid, bias=ub3_sb[:])
                        if debug:
                            nc.gpsimd.dma_start(dbg_u[:], u_row[:])
                        for ch in range(8):
                            ubp = mps.tile([P, 512], F32, space="PSUM", tag="ubp")
                            nc.tensor.matmul(ubp[:], onesr_r[:],
                                             u_row[0:1, ch * 512:(ch + 1) * 512],
                                             start=True, stop=True)
                            for ct in range(2):
                                ot = msc.tile([P, 512], F32, tag="ot")
                                nc.vector.scalar_tensor_tensor(
                                    out=ot[:], in0=ubp[:], scalar=1.0,
                                    in1=fea_sb[:, ct * N + ch * 512:
                                               ct * N + (ch + 1) * 512],
                                    op0=OP.add, op1=OP.mult)
                                (nc.sync if ct == 0 else nc.scalar).dma_start(
                                    out_d[ct * P:(ct + 1) * P,
                                          ch * 512:(ch + 1) * 512],
                                    ot[:])

    nc.compile()
    return nc


def _get_nc(reps=1, debug=False):
    key = (reps, debug)
    if key not in _cache:
        _cache[key] = _build(reps=reps, debug=debug)
    return _cache[key]


def _common_maps(inputs):
    return {
        'Wd': np.ascontiguousarray(
            np.stack([inputs['W1'], inputs['W2'], inputs['W3']]).astype(np.float32)),
        'bd': np.ascontiguousarray(
            np.stack([inputs['b1'], inputs['b2'], inputs['b3']]).astype(np.float32)),
        'U1d': np.ascontiguousarray(np.asarray(inputs['U1'], np.float32)),
        'U2d': np.ascontiguousarray(np.asarray(inputs['U2'], np.float32)),
        'U3d': np.ascontiguousarray(np.asarray(inputs['U3'], np.float32)),
        'ub1d': np.ascontiguousarray(np.asarray(inputs['ub1'], np.float32)),
        'ub2d': np.ascontiguousarray(np.asarray(inputs['ub2'], np.float32)),
        'ub3d': np.ascontiguousarray(np.asarray(inputs['ub3'], np.float32)),
    }


def _in_maps(inputs):
    fea = np.ascontiguousarray(np.asarray(inputs['fea'], dtype=np.float32))
    common = _common_maps(inputs)
    maps = []
    for k in range(NCORES):
        m = dict(common)
        m['fea_own'] = np.ascontiguousarray(fea[k].reshape(C, N))
        m['fea_all'] = np.ascontiguousarray(fea.reshape(B * C, N))
        maps.append(m)
    return maps


def _get_runner(nc):
    """Build the sharded jit callable once and reuse it across kernel() calls."""
    key = id(nc)
    if key in _runner_cache:
        return _runner_cache[key]
    import jax
    from jax.sharding import Mesh, PartitionSpec, NamedSharding
    from jax.experimental.shard_map import shard_map
    from concourse import bass2jax
    bass2jax.install_neuronx_cc_hook()

    partition_name = nc.partition_id_tensor.name if nc.partition_id_tensor else None
    in_names, out_names, out_avals, zero_outs = [], [], [], []
    for alloc in nc.m.functions[0].allocations:
        if not isinstance(alloc, mybir.MemoryLocationSet):
            continue
        name = alloc.memorylocations[0].name
        if alloc.kind == "ExternalInput":
            if name != partition_name:
                in_names.append(name)
        elif alloc.kind == "ExternalOutput":
            shape = tuple(alloc.tensor_shape)
            dtype = mybir.dt.np(alloc.dtype)
            out_names.append(name)
            out_avals.append(jax.core.ShapedArray(shape, dtype))
            zero_outs.append(np.zeros(shape, dtype))
    n_params = len(in_names)
    all_names = in_names + out_names + ([partition_name] if partition_name else [])

    def _body(*args):
        operands = list(args)
        if partition_name is not None:
            operands.append(bass2jax.partition_id_tensor())
        return tuple(bass2jax._bass_exec_p.bind(
            *operands, out_avals=tuple(out_avals),
            in_names=tuple(all_names),
            out_names=tuple(out_names),
            lowering_input_output_aliases=(),
            sim_require_finite=True, sim_require_nnan=True, nc=nc))

    devices = jax.devices()[:NCORES]
    mesh = Mesh(np.asarray(devices), ("core",))
    n_outs = len(out_avals)
    in_specs = (PartitionSpec("core"),) * (n_params + n_outs)
    out_specs = (PartitionSpec("core"),) * n_outs
    sharded = jax.jit(shard_map(_body, mesh=mesh, in_specs=in_specs,
                                out_specs=out_specs, check_rep=False),
                      keep_unused=True)
    sh = NamedSharding(mesh, PartitionSpec("core"))
    zeros_dev = [jax.device_put(np.zeros((NCORES * z.shape[0], *z.shape[1:]), z.dtype), sh)
                 for z in zero_outs]
    runner = {
        'jax': jax, 'sharded': sharded, 'sh': sh,
        'in_names': in_names, 'out_names': out_names,
        'out_avals': out_avals, 'zeros_dev': zeros_dev,
    }
    _runner_cache[key] = runner
    return runner


def kernel(**inputs):
    nc = _get_nc(reps=1)
    r = _get_runner(nc)
    jax = r['jax']
    fea = np.asarray(inputs['fea'], dtype=np.float32)
    common = _common_maps(inputs)
    concat_in = []
    for nm in r['in_names']:
        if nm == 'fea_own':
            concat_in.append(np.ascontiguousarray(fea.reshape(NCORES * C, N)))
        elif nm == 'fea_all':
            full = fea.reshape(B * C, N)
            concat_in.append(np.ascontiguousarray(
                np.broadcast_to(full[None], (NCORES, B * C, N)).reshape(
                    NCORES * B * C, N)))
        else:
            a = common[nm]
            concat_in.append(np.ascontiguousarray(
                np.broadcast_to(a[None], (NCORES, *a.shape)).reshape(
                    NCORES * a.shape[0], *a.shape[1:])))
    dev_in = [jax.device_put(a, r['sh']) for a in concat_in]
    outs = r['sharded'](*dev_in, *r['zeros_dev'])
    oi = r['out_names'].index('out')
    out = np.asarray(outs[oi]).reshape(NCORES, C, N)
    return out.reshape(B, C, HH, WW).astype(np.float32)
